# revision 90
# baseline (speedup 1.0000x reference)
"""Multi-head causal self-attention (B=2, T=4096, C=512, H=8) on 8 trn2 cores.

Sharding: 16 (batch, head) pairs -> 2 heads per core. Core c handles batch
c//4, heads {2*(c%4), 2*(c%4)+1}. All matmuls run in bf16 (1 cycle/row at
any free size), so causal trimming is 128-block granular and the PV matmul
is flipped: the exp'd scores [k, q] are the stationary operand while V
streams only 64+1 columns per k-block (the +1 is an all-ones column matmul
accumulating the softmax row-sums). Per q-chunk, diagonal k-blocks are
processed first so their triangular masks (Pool, SBUF-only) stay off the PE
critical path, and a deep software pipeline (4 score banks, 8-deep PV
queue) hides the exp latency. exp is split between ACT (true exp) and DVE
(Schraudolph bit-trick: bf16 bits = int16(s*128*log2e + 16253.5), ~1%
error), load-balanced by an emission-time cost ledger. Normalization:
DVE reciprocal of the row sums, ACT/DVE staging copy, Pool applies the
scale. o [q,d] -> [d,q] via PE transpose for the row-sliced output
projection; per-chunk partials stream out in bf16 and the host sums the 4
partials per batch, folding the V-bias and output bias in once per batch.
GPSIMD cannot touch PSUM, so every PSUM-reading op lives on ACT/DVE only.
"""

import numpy as np
import ml_dtypes

import concourse.bass as bass
import concourse.mybir as mybir
import concourse.tile as tile
from concourse import bacc
from concourse.bass_utils import run_bass_kernel_spmd

B, T, C, H, D = 2, 4096, 512, 8, 64
NCORES = 8
SCALE = 1.0 / np.sqrt(D)

F32 = mybir.dt.float32
BF16 = mybir.dt.bfloat16
I16 = mybir.dt.int16
BF = ml_dtypes.bfloat16

# Schraudolph exp in bf16 bits: i16 = round(s*A + Bc); bf16(i16) ~ exp(s)
SCH_A = 128.0 / np.log(2.0)
SCH_B = 127.0 * 128.0 - 2.5

TRACE = False
LAST_RESULT = None
_NC = None

_ACT, _DVE, _POOL = "act", "dve", "pool"
USE_POOL_EXP = True


def _build():
    nc = bacc.Bacc()

    xt = nc.declare_dram_parameter("xt", [4, 128, T], BF16, isOutput=False)
    wq = nc.declare_dram_parameter("wq", [4, 128, 128], BF16, isOutput=False)
    wk = nc.declare_dram_parameter("wk", [4, 128, 128], BF16, isOutput=False)
    wv = nc.declare_dram_parameter("wv", [4, 128, 128], BF16, isOutput=False)
    wout = nc.declare_dram_parameter("wout", [128, 4, 128], BF16,
                                     isOutput=False)
    sblob = nc.declare_dram_parameter("sblob", [128, 256], BF16,
                                      isOutput=False)
    fblob = nc.declare_dram_parameter("fblob", [128, 2], F32, isOutput=False)
    out_t = nc.declare_dram_parameter("out_t", [C, T], BF16, isOutput=True)

    busy = {_ACT: 0.0, _DVE: 0.0, _POOL: 0.0}
    RATE = {_ACT: 0.8333, _DVE: 1.0417, _POOL: 1.39}
    FIX = {_ACT: 185.0, _DVE: 125.0, _POOL: 131.0}

    def ledger(eng, cols):
        busy[eng] += cols * RATE[eng] + FIX[eng]

    def pick(cols, engines):
        return min(engines, key=lambda e: busy[e] + cols * RATE[e] + FIX[e])

    with tile.TileContext(nc) as tc:
        with (
            tc.tile_pool(name="w", bufs=1) as w,
            tc.tile_pool(name="sb", bufs=4) as sb,
            tc.tile_pool(name="sbA", bufs=10) as sbA,
            tc.tile_pool(name="psS", bufs=5, space="PSUM") as psS,
            tc.tile_pool(name="psO", bufs=1, space="PSUM") as psO,
            tc.tile_pool(name="psX", bufs=2, space="PSUM") as psX,
        ):
            # ---- persistent tiles ----
            wq_s = w.tile([128, 4, 128], BF16)
            wk_s = w.tile([128, 4, 128], BF16)
            wv_s = w.tile([128, 4, 128], BF16)
            wout_s = w.tile([128, 4, 128], BF16)
            sblob_s = w.tile([128, 256], BF16)
            mask_s = sblob_s[:, 0:128]
            ones_s = sblob_s[:, 127:128]
            ident_s = sblob_s[:, 128:256]
            fblob_s = w.tile([128, 2], F32)
            qb_s = fblob_s[:, 0:1]
            kb_s = fblob_s[:, 1:2]

            xt_s = w.tile([128, 4, T], BF16)
            qt_s = w.tile([128, T], BF16)  # partitions: [h0 dims | h1 dims]
            kt_s = w.tile([128, T], BF16)
            # V in [k, d] layout per 128-block, both heads side by side
            v_all = w.tile([128, 32, 128], BF16)

            # ---- engine dispatch helpers ----
            def emit_exp(eng, at_s, sc_ps, c0, c1):
                cols = c1 - c0
                if eng == _ACT:
                    nc.scalar.activation(
                        at_s[:, c0:c1], sc_ps[:, c0:c1],
                        mybir.ActivationFunctionType.Exp,
                    )
                else:
                    e = nc.vector if eng == _DVE else nc.gpsimd
                    e.tensor_scalar(
                        at_s.bitcast(I16)[:, c0:c1], sc_ps[:, c0:c1],
                        SCH_A, SCH_B,
                        mybir.AluOpType.mult, mybir.AluOpType.add,
                    )
                ledger(eng, cols)

            def emit_mask(eng, at_s, c0):
                e = nc.vector if eng == _DVE else nc.gpsimd
                e.tensor_tensor(
                    at_s[:, c0:c0 + 128], at_s[:, c0:c0 + 128],
                    mask_s, mybir.AluOpType.mult,
                )
                ledger(eng, 128)

            def copy_op(dst, src, cols, scalar=None, engines=(_DVE, _ACT)):
                # NOTE: Pool/GPSIMD cannot access PSUM; psum-reading copies
                # may only run on ACT (plain Copy) or DVE (tensor_scalar).
                eng = pick(cols, engines)
                e = {_DVE: nc.vector, _POOL: nc.gpsimd,
                     _ACT: nc.scalar}[eng]
                if eng == _ACT:
                    assert scalar is None
                    nc.scalar.activation(dst, src,
                                         mybir.ActivationFunctionType.Copy)
                elif scalar is None:
                    e.tensor_copy(dst, src)
                else:
                    e.tensor_scalar(dst, src, scalar, None,
                                    mybir.AluOpType.mult)
                ledger(eng, cols)

            # ---- projection pieces (drip-fed under attention) ----
            def proj_qk(g, which):
                ws, dst, bias = ((wq_s, qt_s, qb_s) if which == "q"
                                 else (wk_s, kt_s, kb_s))
                sl = bass.ts(g, 512)
                pp = psX.tile([128, 512], F32, tag="x")
                for ch in range(4):
                    nc.tensor.matmul(
                        pp, ws[:, ch, :], xt_s[:, ch, sl],
                        start=(ch == 0), stop=(ch == 3),
                    )
                nc.vector.tensor_scalar(
                    dst[:, sl], pp, 1.0, bias,
                    mybir.AluOpType.mult, mybir.AluOpType.add,
                )
                ledger(_DVE, 512)

            def proj_v(g, t4):
                tt = g * 4 + t4
                pp = psX.tile([128, 512], F32, tag="x")
                pv = pp[:, 0:128]
                for ch in range(4):
                    nc.tensor.matmul(
                        pv, xt_s[:, ch, bass.ts(tt, 128)], wv_s[:, ch, :],
                        start=(ch == 0), stop=(ch == 3),
                    )
                copy_op(v_all[:, tt, :], pv, 128)

            proj_pending = []  # (chunk, fn)
            deferred = []

            def queue_proj(g):
                sl = bass.ts(g, 512)
                nc.sync.dma_start(out=xt_s[:, :, sl],
                                  in_=xt.rearrange("c p t -> p c t")[:, :, sl])
                for which in ("q", "k"):
                    proj_pending.append(
                        (g, lambda g=g, w_=which: proj_qk(g, w_)))
                for t4 in range(4):
                    proj_pending.append(
                        (g, lambda g=g, t4=t4: proj_v(g, t4)))

            drip_tick = [0]

            dma_pending = []
            last_chunk = [False]

            def drip():
                drip_tick[0] += 1
                if dma_pending and drip_tick[0] % 2 == 1:
                    dma_pending.pop(0)()
                if proj_pending:
                    proj_pending.pop(0)[1]()
                elif deferred:
                    # drain deferred at half rate to keep outproj matmuls
                    # well behind their transposed inputs (full rate in the
                    # last chunk so the tail doesn't serialize)
                    if last_chunk[0] or drip_tick[0] % 2 == 0:
                        deferred.pop(0)()

            # ---- attention ----
            pv_queue = []

            def attn_segment(g, h, onorm_s):
                if h == 0:
                    while proj_pending and proj_pending[0][0] <= g:
                        proj_pending.pop(0)[1]()
                hb = h * 64
                o_ps = psO.tile([128, 4, 65], F32, tag="o")
                # one PSUM bank cannot host 4 concurrent accumulation groups;
                # zero it once and accumulate group-free instead
                nc.vector.memset(o_ps, 0.0)
                ledger(_DVE, 260)
                # diag blocks first (their masks stay off the critical path)
                js = list(range(4 * g, 4 * g + 4)) + list(range(4 * g))
                # per-qb bookkeeping for PSUM accumulate start/stop
                contrib = {qb: [j for j in js
                                if j < 4 * g or j - 4 * g <= qb]
                           for qb in range(4)}
                first = {qb: contrib[qb][0] for qb in range(4)}
                last = {qb: contrib[qb][-1] for qb in range(4)}

                for j in js:
                    d = j - 4 * g
                    q0 = max(d, 0) * 128
                    cols = 512 - q0
                    sc_ps = psS.tile([128, 512], F32, tag="sc")
                    at_s = sbA.tile([128, 512], BF16, tag="attn")
                    nc.tensor.matmul(
                        sc_ps[:, q0:512],
                        kt_s[hb:hb + 64, bass.ts(j, 128)],
                        qt_s[hb:hb + 64, g * 512 + q0:(g + 1) * 512],
                        start=True, stop=True,
                    )
                    if d >= 0:
                        # diag tiles: exp on ACT or DVE (psum access); the
                        # triangular mask is SBUF-only so it runs on Pool.
                        # Diag-first ordering gives these PVs 4 js of slack,
                        # so the cross-engine hop is fine.
                        eng = pick(cols, (_ACT, _DVE))
                        emit_exp(eng, at_s, sc_ps, q0, 512)
                        emit_mask(_POOL, at_s, q0)
                    else:
                        eng = pick(cols, (_ACT, _DVE))
                        emit_exp(eng, at_s, sc_ps, 0, 512)

                    def pv(j=j, d=d, at_s=at_s, o_ps=o_ps, hb=hb,
                           first=first, last=last):
                        for qb in range(max(d, 0), 4):
                            nc.tensor.matmul(
                                o_ps[:, qb, 0:64],
                                at_s[:, qb * 128:(qb + 1) * 128],
                                v_all[:, j, hb:hb + 64],
                                start=False,
                                stop=(j == last[qb]),
                                skip_group_check=True,
                            )
                            nc.tensor.matmul(
                                o_ps[:, qb, 64:65],
                                at_s[:, qb * 128:(qb + 1) * 128],
                                ones_s,
                                start=False,
                                stop=(j == last[qb]),
                                skip_group_check=True,
                            )
                    pv_queue.append(pv)
                    if len(pv_queue) > 7:
                        pv_queue.pop(0)()
                    drip()

                while pv_queue:
                    pv_queue.pop(0)()
                # normalization: DVE recip (psum), ACT/DVE stage o to SBUF,
                # Pool (SBUF-only) applies the per-row reciprocal
                rec_s = sb.tile([128, 4], F32, tag="rec")
                with nc.allow_low_precision(reason="softmax denom"):
                    nc.vector.reciprocal(rec_s, o_ps[:, :, 64])
                ledger(_DVE, 4)
                osb_s = sb.tile([128, 4, 64], BF16, tag="osb")
                copy_op(osb_s, o_ps[:, :, 0:64], 256, engines=(_ACT, _DVE))
                for qb in range(4):
                    nc.gpsimd.tensor_scalar(
                        onorm_s[:, qb, hb:hb + 64], osb_s[:, qb, :],
                        rec_s[:, qb:qb + 1], None, mybir.AluOpType.mult,
                    )
                    ledger(_POOL, 64)

            def outproj_m(g, onormT_s, oc_s, m):
                op_ps = psX.tile([128, 512], F32, tag="x")
                nc.tensor.matmul(
                    op_ps, wout_s[:, m, :], onormT_s,
                    start=True, stop=True,
                )
                copy_op(oc_s[:, m, :], op_ps, 512, engines=(_ACT, _DVE))
                if last_chunk[0] or m == 3:
                    # one store per chunk normally; per-m in the last chunk
                    # so the final DMAs overlap the remaining copies
                    m0 = m if last_chunk[0] else 0
                    dma_pending.append(
                        lambda g=g, oc_s=oc_s, m0=m0, m=m: nc.sync.dma_start(
                            out=out_t.rearrange(
                                "(m p) t -> p m t",
                                m=4)[:, m0:m + 1, bass.ts(g, 512)],
                            in_=oc_s[:, m0:m + 1, :]))

            # ---- startup: interleave DMAs on both queues, start projecting
            # as soon as the needed chunks land ----
            sl0 = bass.ts(0, 512)
            nc.sync.dma_start(out=xt_s[:, :, sl0],
                              in_=xt.rearrange("c p t -> p c t")[:, :, sl0])
            nc.scalar.dma_start(out=wq_s, in_=wq.rearrange("c p m -> p c m"))
            nc.scalar.dma_start(out=wk_s, in_=wk.rearrange("c p m -> p c m"))
            nc.scalar.dma_start(out=fblob_s, in_=fblob[:])
            nc.scalar.dma_start(out=wv_s, in_=wv.rearrange("c p m -> p c m"))
            nc.scalar.dma_start(out=sblob_s, in_=sblob[:])
            warm_s = sb.tile([1, 1], F32, tag="warm")
            nc.scalar.activation(warm_s, fblob_s[0:1, 0:1],
                                 mybir.ActivationFunctionType.Exp)
            nc.sync.dma_start(out=wout_s, in_=wout[:])
            # PE p-state warmup during the startup DMAs
            wu_s = sb.tile([128, 512], BF16, tag="wu", bufs=1)
            nc.gpsimd.memset(wu_s, 0.0)
            wu_ps = psX.tile([128, 512], F32, tag="x")
            for _ in range(8):
                nc.tensor.matmul(wu_ps, wu_s[:, 0:128], wu_s,
                                 start=True, stop=True)
            for which in ("q", "k"):
                proj_qk(0, which)
            for t4 in range(4):
                proj_v(0, t4)

            for g in range(8):
                if g < 7:
                    queue_proj(g + 1)
                else:
                    last_chunk[0] = True
                onorm_s = sb.tile([128, 4, 128], BF16, tag="onorm")
                onormT_s = sb.tile([128, 512], BF16, tag="onormT")
                oc_s = sb.tile([128, 4, 512], BF16, tag="outc")
                attn_segment(g, 0, onorm_s)
                attn_segment(g, 1, onorm_s)

                # transpose [q,d]->[d,q] on the PE (bf16, 128 cyc each)
                def trans(onorm_s=onorm_s, onormT_s=onormT_s, qb=0):
                    tr_ps = psX.tile([128, 128], BF16, tag="x")
                    nc.tensor.transpose(tr_ps, onorm_s[:, qb, :], ident_s)
                    copy_op(onormT_s[:, qb * 128:(qb + 1) * 128], tr_ps, 128)
                for qb in range(4):
                    deferred.append(
                        lambda onorm_s=onorm_s, onormT_s=onormT_s, qb=qb:
                        trans(onorm_s, onormT_s, qb))
                for m in range(4):
                    deferred.append(
                        lambda g=g, onormT_s=onormT_s, oc_s=oc_s, m=m:
                        outproj_m(g, onormT_s, oc_s, m))
            while pv_queue:
                pv_queue.pop(0)()
            while proj_pending or deferred or dma_pending:
                if proj_pending:
                    proj_pending.pop(0)[1]()
                elif deferred:
                    deferred.pop(0)()
                else:
                    dma_pending.pop(0)()
    nc.compile()
    return nc


def _pack_inputs(x, Wqkv, bqkv, Wout, bout):
    sb_host = np.zeros((128, 256), dtype=np.float32)
    sb_host[:, 0:128] = np.triu(np.ones((128, 128), dtype=np.float32))
    sb_host[:, 128:256] = np.eye(128, dtype=np.float32)
    in_maps = []
    for c in range(NCORES):
        b = c // 4
        h0 = 2 * (c % 4)
        cq = h0 * 64
        xt = np.ascontiguousarray(x[b].T.reshape(4, 128, T))
        wq_c = np.ascontiguousarray(
            (Wqkv[:, cq:cq + 128] * SCALE).reshape(4, 128, 128))
        wk_c = np.ascontiguousarray(
            Wqkv[:, 512 + cq:512 + cq + 128].reshape(4, 128, 128))
        wv_c = np.ascontiguousarray(
            Wqkv[:, 1024 + cq:1024 + cq + 128].reshape(4, 128, 128))
        wout_c = np.ascontiguousarray(
            Wout[cq:cq + 128, :].reshape(128, 4, 128))
        fblob = np.zeros((128, 2), dtype=np.float32)
        fblob[:, 0] = bqkv[cq:cq + 128] * SCALE
        fblob[:, 1] = bqkv[512 + cq:512 + cq + 128]
        in_maps.append({
            "xt": xt.astype(BF),
            "wq": wq_c.astype(BF), "wk": wk_c.astype(BF),
            "wv": wv_c.astype(BF), "wout": wout_c.astype(BF),
            "sblob": sb_host.astype(BF), "fblob": fblob,
        })
    return in_maps


def kernel(x, Wqkv, bqkv, Wout, bout):
    global _NC, LAST_RESULT
    x = np.asarray(x, dtype=np.float32)
    Wqkv = np.asarray(Wqkv, dtype=np.float32)
    bqkv = np.asarray(bqkv, dtype=np.float32)
    Wout = np.asarray(Wout, dtype=np.float32)
    bout = np.asarray(bout, dtype=np.float32)

    if _NC is None:
        _NC = _build()
    in_maps = _pack_inputs(x, Wqkv, bqkv, Wout, bout)
    res = run_bass_kernel_spmd(_NC, in_maps, list(range(NCORES)), trace=TRACE)
    LAST_RESULT = res
    # v-bias contribution (sum_k attn = 1) + output bias, applied once/batch
    base = (bqkv[1024:] @ Wout + bout).astype(np.float32)
    out = np.zeros((B, T, C), dtype=np.float32)
    out += base
    for c in range(NCORES):
        out[c // 4] += res.results[c]["out_t"].astype(np.float32).T
    return out
